# revision 13
# baseline (speedup 1.0000x reference)
"""Trainium2 Bass kernel for nn_DemandRouter (retrieval kNN).

Reference computation (per batch b):
    Q = x @ Wq.T + bq          [T, 32]
    K = x @ Wk.T + bk          [T, 32]
    sim = Q @ K.T / sqrt(32)   [T, T]
    idx = top_k(sim, 4)        [T, 4]
    out[t] = mean(x[idx[t]])   [T, D]

Sharding: 8 cores = 4 batches x 2 T-halves (data parallel over B, then
split T). The kernel is DMA-bandwidth-bound (~330 GB/s/core), so the
design minimizes HBM bytes:

  - The host passes x[b] transposed (no on-device transposes) and only
    the core's OWN T-half of it (4 MiB): each core of a batch-pair
    projects keys for its half, then the two cores exchange biased
    K^T halves (128 KiB) with an intra-pair AllGather. This halves the
    x-load versus each core projecting all 2048 keys.
  - The 1/sqrt(32) sim scale is dropped (argmax-invariant).
  - Top-4 comes from the DVE max/max_index top-8 unit reading the sim
    PSUM tile directly (no PSUM->SBUF copy of sim).
  - The 4-neighbor mean is built by indirect-DMA gathers with on-DMA
    accumulate (cce add) in pairs + one DVE add + ACT 0.25 scale.

Per-core pipeline:
  W. ~4us of dummy matmuls ramp the PE p-state under the first DMA.
  A. stream xth d-row tiles [128, 1024]; accumulate Wqk^T.T @ xth into
     PSUM -> [Q;K]^T of own half [64, 1024] (contract d, 8 chunks).
  B. PSUM -> SBUF with per-partition bias (ScalarE); ship K^T half to
     DRAM, AllGather pair halves, reload full K^T [32, 2048].
  C. per 128-row t-tile: sim = Q^T.T @ K^T into a 4-bank PSUM tile
     [128, 2048]; DVE max/max_index -> top-8 values+indices.
  D. 4 indirect-DMA gathers of x rows (pairs accumulated on the DMA),
     1 add + 0.25 scale; store the 128x1024 output tile.
"""

import os

import numpy as np

import concourse.bass as bass
import concourse.mybir as mybir
import concourse.tile as tile
from concourse import bacc
from concourse.bass import ts
from concourse.bass_utils import run_bass_kernel_spmd

B, T, D = 4, 2048, 1024
KQ = 32          # query/key projection width
KTOP = 4
P = 128
N_CORES = 8
TQ = T // 2      # query rows handled per core
ND = D // P      # 8 contraction chunks of 128
NG = 4           # t column-groups of full T
GT = T // NG     # 512 t per group
NGH = 2          # t column-groups of own half
NT = TQ // P     # 8 query row-tiles per core

f32 = mybir.dt.float32
f32r = mybir.dt.float32r
u32 = mybir.dt.uint32
IDENT = mybir.ActivationFunctionType.Identity

# experiment flags (read at module build time)
USE_F32R = os.environ.get("KERNEL_F32R", "0") == "1"
USE_CCE = os.environ.get("KERNEL_CCE", "1") == "1"
USE_PAIR = os.environ.get("KERNEL_PAIR", "1") == "1"
ABLATE = os.environ.get("KERNEL_ABLATE", "")

# float32r is *rounded* fp32 (reduced precision) — measured 0.025 rel err
# on this problem, so it stays off; exact fp32 everywhere.
MM_DT = f32r if USE_F32R else f32

PAIR_GROUPS = [[0, 1], [2, 3], [4, 5], [6, 7]]

_NC = None


def _emit_warmup(tc, nc):
    from contextlib import ExitStack

    # ~4us of dummy matmuls so the PE p-state ramps to 2.4 GHz while the
    # first input DMA is in flight. Pools scoped so the PSUM bank frees
    # before phase C needs all 8.
    with ExitStack() as wctx:
        wu = wctx.enter_context(tc.tile_pool(name="wu", bufs=1))
        wups = wctx.enter_context(tc.tile_pool(name="wups", bufs=1, space="PSUM"))
        wsb = wu.tile([P, P], f32)
        nc.gpsimd.memset(wsb[:], 1.0)
        wps = wups.tile([P, P], f32)
        for _ in range(10):
            nc.tensor.matmul(wps[:], lhsT=wsb[:], rhs=wsb[:], start=True, stop=True)


def _emit_topk_gather(tc, nc, pcd, qt, kt, xg, out):
    """Phases C+D: sim, top-k, gather, mean, store."""
    psim = pcd.enter_context(tc.tile_pool(name="psim", bufs=2, space="PSUM"))
    gpool = pcd.enter_context(tc.tile_pool(name="gpool", bufs=2))
    mpool = pcd.enter_context(tc.tile_pool(name="mpool", bufs=3))
    opool = pcd.enter_context(tc.tile_pool(name="opool", bufs=2))

    for i in range(NT):
        simp = psim.tile([P, T], f32, tag="sim", name=f"sim{i}")
        for c in range(NG):
            nc.tensor.matmul(
                simp[:, ts(c, GT)],
                lhsT=qt[:, ts(i, P)],
                rhs=kt[:, ts(c, GT)],
                start=True,
                stop=True,
            )
        mx = mpool.tile([P, 8], f32, tag="mx", name=f"mx{i}")
        ix = mpool.tile([P, 8], u32, tag="ix", name=f"ix{i}")
        nc.vector.max(out=mx[:], in_=simp[:])
        nc.vector.max_index(out=ix[:], in_max=mx[:], in_values=simp[:])

        if ABLATE == "nogather":
            g = [
                gpool.tile([P, D], f32, tag=f"g{k}", name=f"g{k}_{i}")
                for k in range(2)
            ]
            nc.gpsimd.memset(g[0][:], 0.5)
            nc.gpsimd.memset(g[1][:], 0.25)
            s01 = opool.tile([P, D], f32, tag="s01", name=f"s01_{i}")
            nc.vector.tensor_add(s01[:], g[0][:], g[1][:])
        elif USE_CCE:
            g = [
                gpool.tile([P, D], f32, tag=f"g{k}", name=f"g{k}_{i}")
                for k in range(2)
            ]
            for k in range(KTOP):
                nc.gpsimd.indirect_dma_start(
                    out=g[k % 2][:],
                    out_offset=None,
                    in_=xg[:, :],
                    in_offset=bass.IndirectOffsetOnAxis(ap=ix[:, k : k + 1], axis=0),
                    compute_op=(
                        mybir.AluOpType.add if k >= 2 else mybir.AluOpType.bypass
                    ),
                )
            s01 = opool.tile([P, D], f32, tag="s01", name=f"s01_{i}")
            nc.vector.tensor_add(s01[:], g[0][:], g[1][:])
        else:
            g = [
                gpool.tile([P, D], f32, tag=f"g{k}", name=f"g{k}_{i}")
                for k in range(KTOP)
            ]
            for k in range(KTOP):
                nc.gpsimd.indirect_dma_start(
                    out=g[k][:],
                    out_offset=None,
                    in_=xg[:, :],
                    in_offset=bass.IndirectOffsetOnAxis(ap=ix[:, k : k + 1], axis=0),
                )
            s01 = opool.tile([P, D], f32, tag="s01", name=f"s01_{i}")
            s23 = opool.tile([P, D], f32, tag="s23", name=f"s23_{i}")
            nc.vector.tensor_add(s01[:], g[0][:], g[1][:])
            nc.vector.tensor_add(s23[:], g[2][:], g[3][:])
            nc.vector.tensor_add(s01[:], s01[:], s23[:])
        ot = opool.tile([P, D], f32, tag="ot", name=f"ot{i}")
        nc.scalar.mul(ot[:], s01[:], 0.25)
        nc.sync.dma_start(out[ts(i, P), :], ot[:])


def _emit_pair(tc, nc, xg, xth, wqkt, bqk, out, warmup):
    """Pair-sharing variant: project own T-half only, AllGather K^T.

    Everything is in GLOBAL coordinates: sim columns are global t, the
    gather table xg is the unrolled x[b], and the output rows are the
    core's own global query rows.
    """
    from contextlib import ExitStack

    with ExitStack() as ctx:
        if warmup:
            _emit_warmup(tc, nc)
        cpool = ctx.enter_context(tc.tile_pool(name="consts", bufs=1))
        wq_sb = cpool.tile([P, ND, 2 * KQ], MM_DT)  # [128, 8, 64]; d = dd*128+p
        nc.sync.dma_start(wq_sb[:], wqkt.rearrange("(n p) k -> p n k", p=P))
        bqk_sb = cpool.tile([2 * KQ, 1], f32)
        nc.sync.dma_start(bqk_sb[:], bqk[:])
        qt = cpool.tile([KQ, TQ], f32)  # Q^T (own half) with bias
        kt = cpool.tile([KQ, NGH, TQ], f32)  # K^T (full T) with bias

        dpool = ctx.enter_context(tc.tile_pool(name="ccdram", bufs=1, space="DRAM"))
        cc_in = dpool.tile([KQ, TQ], f32)
        cc_out = dpool.tile([2 * KQ, TQ], f32)

        # ---- phase A: load own xth half + project ----
        with ExitStack() as pa:
            xt_pool = pa.enter_context(tc.tile_pool(name="xt", bufs=3))
            pqkt = pa.enter_context(tc.tile_pool(name="pqkt", bufs=1, space="PSUM"))
            qk_ps = [
                pqkt.tile([2 * KQ, GT], f32, tag=f"qk{c}", name=f"qk_ps{c}")
                for c in range(NGH)
            ]
            kth = cpool.tile([KQ, TQ], f32)  # own biased K^T half
            if ABLATE == "noproj":
                nc.vector.memset(qt[:], 0.001)
                nc.vector.memset(kth[:], 0.002)
            for dd in range(ND if ABLATE != "noproj" else 0):
                xt = xt_pool.tile([P, TQ], MM_DT, tag="xt", name=f"xt{dd}")
                nc.sync.dma_start(xt[:], xth[ts(dd, P), :])
                for c in range(NGH):
                    nc.tensor.matmul(
                        qk_ps[c][:],
                        lhsT=wq_sb[:, dd, :],
                        rhs=xt[:, ts(c, GT)],
                        start=(dd == 0),
                        stop=(dd == ND - 1),
                    )

            # ---- phase B: PSUM -> SBUF with bias ----
            for c in range(NGH if ABLATE != "noproj" else 0):
                nc.scalar.activation(
                    qt[:, ts(c, GT)], qk_ps[c][0:KQ, :], IDENT, bias=bqk_sb[0:KQ, :]
                )
                nc.scalar.activation(
                    kth[:, ts(c, GT)],
                    qk_ps[c][KQ : 2 * KQ, :],
                    IDENT,
                    bias=bqk_sb[KQ : 2 * KQ, :],
                )
        nc.sync.dma_start(cc_in[:], kth[:])
        nc.gpsimd.collective_compute(
            "AllGather",
            mybir.AluOpType.bypass,
            replica_groups=PAIR_GROUPS,
            ins=[cc_in[:]],
            outs=[cc_out[:]],
        )
        # cc_out rows [0:32] = pair rank 0 (global t 0..1023), rows
        # [32:64] = pair rank 1 — global column order for both cores.
        nc.sync.dma_start(kt[:], cc_out.rearrange("(h k) s -> k h s", k=KQ))

        with ExitStack() as pcd:
            _emit_topk_gather(
                tc, nc, pcd, qt, kt.rearrange("k h s -> k (h s)"), xg, out
            )


def _emit_solo(tc, nc, xg, xrt, wqkt, bqk, out, warmup):
    """Original variant: every core projects all T keys itself (rolled
    coordinates: the core's queries are rows [0:1024) of the rolled x)."""
    from contextlib import ExitStack

    with ExitStack() as ctx:
        if warmup:
            _emit_warmup(tc, nc)
        cpool = ctx.enter_context(tc.tile_pool(name="consts", bufs=1))
        wq_sb = cpool.tile([P, ND, 2 * KQ], MM_DT)
        nc.sync.dma_start(wq_sb[:], wqkt.rearrange("(n p) k -> p n k", p=P))
        bqk_sb = cpool.tile([2 * KQ, 1], f32)
        nc.sync.dma_start(bqk_sb[:], bqk[:])
        qt = cpool.tile([KQ, T], f32)
        kt = cpool.tile([KQ, T], f32)

        with ExitStack() as pa:
            xt_pool = pa.enter_context(tc.tile_pool(name="xt", bufs=3))
            pqkt = pa.enter_context(tc.tile_pool(name="pqkt", bufs=1, space="PSUM"))
            qk_ps = [
                pqkt.tile([2 * KQ, GT], f32, tag=f"qk{c}", name=f"qk_ps{c}")
                for c in range(NG)
            ]
            if ABLATE == "noproj":
                nc.vector.memset(qt[:], 0.001)
                nc.vector.memset(kt[:], 0.002)
            for dd in range(ND if ABLATE != "noproj" else 0):
                xt = xt_pool.tile([P, T], MM_DT, tag="xt", name=f"xt{dd}")
                nc.sync.dma_start(xt[:], xrt[ts(dd, P), :])
                for c in range(NG):
                    nc.tensor.matmul(
                        qk_ps[c][:],
                        lhsT=wq_sb[:, dd, :],
                        rhs=xt[:, ts(c, GT)],
                        start=(dd == 0),
                        stop=(dd == ND - 1),
                    )
            for c in range(NG if ABLATE != "noproj" else 0):
                nc.scalar.activation(
                    qt[:, ts(c, GT)], qk_ps[c][0:KQ, :], IDENT, bias=bqk_sb[0:KQ, :]
                )
                nc.scalar.activation(
                    kt[:, ts(c, GT)],
                    qk_ps[c][KQ : 2 * KQ, :],
                    IDENT,
                    bias=bqk_sb[KQ : 2 * KQ, :],
                )

        with ExitStack() as pcd:
            _emit_topk_gather(tc, nc, pcd, qt, kt, xg, out)


def _build_module():
    repeat = int(os.environ.get("KERNEL_REPEAT", "1"))
    nc = bacc.Bacc(
        "TRN2", target_bir_lowering=False, debug=False, num_devices=N_CORES
    )
    if USE_PAIR:
        xg = nc.dram_tensor("xg", [T, D], f32, kind="ExternalInput").ap()
        xth = nc.dram_tensor("xth", [D, TQ], MM_DT, kind="ExternalInput").ap()
        wqkt = nc.dram_tensor("wqkt", [D, 2 * KQ], MM_DT, kind="ExternalInput").ap()
        bqk = nc.dram_tensor("bqk", [2 * KQ, 1], f32, kind="ExternalInput").ap()
        out = nc.dram_tensor("out", [TQ, D], f32, kind="ExternalOutput").ap()
        with tile.TileContext(nc) as tc:
            for r in range(repeat):
                _emit_pair(tc, nc, xg, xth, wqkt, bqk, out, warmup=(r == 0))
    else:
        xg = nc.dram_tensor("xr", [T, D], f32, kind="ExternalInput").ap()
        xrt = nc.dram_tensor("xrt", [D, T], MM_DT, kind="ExternalInput").ap()
        wqkt = nc.dram_tensor("wqkt", [D, 2 * KQ], MM_DT, kind="ExternalInput").ap()
        bqk = nc.dram_tensor("bqk", [2 * KQ, 1], f32, kind="ExternalInput").ap()
        out = nc.dram_tensor("out", [TQ, D], f32, kind="ExternalOutput").ap()
        with tile.TileContext(nc) as tc:
            for r in range(repeat):
                _emit_solo(tc, nc, xg, xrt, wqkt, bqk, out, warmup=(r == 0))
    nc.compile()
    return nc


def _get_nc():
    global _NC
    if _NC is None:
        _NC = _build_module()
    return _NC


def _make_in_maps(x, Wq, bq, Wk, bk):
    x = np.ascontiguousarray(np.asarray(x, dtype=np.float32))
    wqkt = np.ascontiguousarray(
        np.concatenate(
            [np.asarray(Wq, np.float32).T, np.asarray(Wk, np.float32).T], axis=1
        )
    )
    bqk = np.concatenate(
        [np.asarray(bq, np.float32), np.asarray(bk, np.float32)]
    )[:, None]
    bqk = np.ascontiguousarray(bqk)
    in_maps = []
    for c in range(N_CORES):
        b, h = divmod(c, 2)
        off = h * TQ
        xb = x[b]
        if USE_PAIR:
            in_maps.append(
                {
                    "xg": xb,
                    "xth": np.ascontiguousarray(xb[off : off + TQ].T),
                    "wqkt": wqkt,
                    "bqk": bqk,
                }
            )
        else:
            xrc = np.concatenate([xb[off:], xb[:off]], axis=0) if off else xb
            in_maps.append(
                {
                    "xr": np.ascontiguousarray(xrc),
                    "xrt": np.ascontiguousarray(xrc.T),
                    "wqkt": wqkt,
                    "bqk": bqk,
                }
            )
    return in_maps


def run(x, Wq, bq, Wk, bk, trace=False):
    """Run on 8 cores; returns (full_output, BassKernelResults)."""
    in_maps = _make_in_maps(x, Wq, bq, Wk, bk)
    nc = _get_nc()
    res = run_bass_kernel_spmd(nc, in_maps, list(range(N_CORES)), trace=trace)
    outf = np.empty((B, T, D), np.float32)
    for c in range(N_CORES):
        b, h = divmod(c, 2)
        outf[b, h * TQ : (h + 1) * TQ] = res.results[c]["out"]
    return outf, res


def kernel(x, Wq, bq, Wk, bk):
    outf, _ = run(x, Wq, bq, Wk, bk, trace=False)
    return outf
